# revision 7
# baseline (speedup 1.0000x reference)
"""MRU encoding kernel for Trainium2 (8 NeuronCores, batch-parallel).

Problem (B=32, T=2048, D=300):
    z = tanh(x @ Wz.T + bz); o = tanh(x @ Wo.T + bo)
    c_t = g_t*c_{t-1} + (1-g_t)*z_t   (c_{-1}=0, scan over T)
    out = o * c

Per-core (4 batch rows) layout is [channel, time]:
  - host pre-transposes x,g to [b, D, T]; x gets a ones-row (301) so the
    bias rides in the matmul contraction; the two weight matrices are fed
    as ONE combined [D+1, 5, 128] operand: slices 0,1 = Wz.T columns
    0:256, slices 2,3 = Wo.T columns 0:256, slice 4 = the ragged columns
    of BOTH weights ([Wz.T 256:300 | zeros | Wo.T 256:300 | zeros]) so
    one GEMM per (row, slice) covers them: 15 matmul groups per row
    instead of 18.
  - o is produced NEGATED via tanh(scale=-1): with bneg=(g-1)*z = -(1-g)z
    the hardware scan state=g*state+bneg yields -c, and (-o)*(-c) = o*c.
  - the whole T=2048 recurrence per channel is ONE tensor_tensor_scan
    DVE instruction per 128-channel tile (state kept fp32 by HW).
  - bneg is computed on the (otherwise idle) GPSIMD engine via
    scalar_tensor_tensor so the DVE only runs scans and the final mult.
  - the ragged slice-4 psum holds z at partitions 0:44 and o at 64:108;
    partition-shifted activations repack BOTH batch rows of a pair into
    one 128-lane z2/oneg2 pair (b_even at 0:44, b_odd at 64:108) so the
    ragged recurrence is one DVE chain per pair.  The zero weight-pad
    columns make the activations also rewrite the pad lanes (tanh(0)=0),
    keeping every lane finite without extra memsets.
  - input loads ride the SP HWDGE ring; weights+stores ride the ACT ring
    (HWDGE is FIFO per issuing engine; stores must not block prefetch).
"""

import numpy as np

import concourse.bass as bass
import concourse.mybir as mybir
import concourse.tile as tile
from concourse import bacc
from concourse.bass_utils import run_bass_kernel_spmd

B, T, D = 32, 2048, 300
NCORES = 8
BC = B // NCORES  # 4 batch rows per core
DP = D + 1  # ones-row at index 300 carries the bias
NS = 5  # combined-weight m-slices: z0 z1 o0 o1 ragged
TS = 512  # moving-operand max free dim
NT = T // TS
F32 = mybir.dt.float32
F32R = mybir.dt.float32r
F16 = mybir.dt.float16

KC = [(0, 128), (128, 128), (256, 45)]  # k-chunks (incl. ones row)

CFG = {"mm16": True, "plane16": True, "c16": True, "out16": True}

_CACHE: dict = {}

Tanh = mybir.ActivationFunctionType.Tanh


def _build_program(reps=1, bufs=None, cfg=None):
    c = dict(CFG)
    if cfg:
        c.update(cfg)
    mm_dt = F16 if c["mm16"] else F32R
    pl_dt = F16 if c["plane16"] else F32
    c_dt = F16 if c["c16"] else F32
    out_dt = F16 if c["out16"] else F32

    bf = {"xp": 2, "gp": 2, "zp": 2, "ep": 3, "ps": 2}
    if bufs:
        bf.update(bufs)

    nc = bacc.Bacc("TRN2", target_bir_lowering=False, debug=False, num_devices=NCORES)

    d_x = nc.dram_tensor("xt", [BC, DP, T], mm_dt, kind="ExternalInput").ap()
    d_g = nc.dram_tensor("gt", [BC, D, T], pl_dt, kind="ExternalInput").ap()
    d_w = nc.dram_tensor("wzo", [DP, NS * 128], mm_dt, kind="ExternalInput").ap()
    # replicas share ONE output tensor: keeps the PJRT buffer count (and its
    # per-call overhead) constant across reps so marginal timing is clean
    d_out0 = nc.dram_tensor("outt", [BC, D, T], out_dt, kind="ExternalOutput").ap()
    d_outs = [d_out0] * reps

    with tile.TileContext(nc) as tc:
        with (
            tc.tile_pool(name="wp", bufs=1) as wp,
            tc.tile_pool(name="g2p", bufs=1) as g2p,
            tc.tile_pool(name="xp", bufs=bf["xp"]) as xp,
            tc.tile_pool(name="gp", bufs=bf["gp"]) as gp,
            tc.tile_pool(name="zp", bufs=bf["zp"]) as zp,
            tc.tile_pool(name="ep", bufs=bf["ep"]) as ep,
            tc.tile_pool(name="ps", bufs=bf["ps"], space="PSUM") as ps,
        ):
            # weights ride the scalar ring so they don't delay the first x load
            w = wp.tile([128, 3, NS, 128], mm_dt, tag="w", name="w_t")
            nc.scalar.dma_start(
                w[:, 0:2, :, :],
                d_w[0:256, :].rearrange("(c p) (s m) -> p c s m", c=2, s=NS),
            )
            nc.scalar.dma_start(
                w[:45, 2, :, :], d_w[256:DP, :].rearrange("p (s m) -> p s m", s=NS)
            )

            # persistent ragged-gate tiles (one per pair): pad lanes memset
            # once to a scan-safe finite value, live lanes DMA'd per pair
            g2s = []
            for pr in range(BC // 2):
                g2 = g2p.tile([128, T], pl_dt, tag=f"g2_{pr}", name=f"g2_{pr}")
                nc.gpsimd.memset(g2[32:64, :], 0.5)
                nc.gpsimd.memset(g2[96:128, :], 0.5)
                g2s.append(g2)

            # A store dma_start holds the ACT SEQ while its data-ready sem is
            # pending (cost model: waits precede free(SEQ)), starving the
            # activations behind it.  Deferring each chain's stores until the
            # next chain is issued makes every wait resolve instantly.
            pending_stores: list = []

            def flush_stores():
                for ds, res_ap in pending_stores:
                    # stores ride the ACT HWDGE ring (never block prefetch)
                    nc.scalar.dma_start(ds, res_ap)
                pending_stores.clear()

            def chain(gs, z_ap, oneg_ap, stores, tsplit=1, bneg_dve=False):
                """bneg=(g-1)z -> scan(-c) -> out = (-o)*(-c); stores is a
                list of (res_slice, dram_slice). tsplit>1 pipelines the chain
                in T-chunks (scan chained via `initial`) so the final store
                overlaps the rest -- used for the kernel-tail chain."""
                bneg = ep.tile([128, T], pl_dt, tag="bneg", name="bneg_t")
                cneg = ep.tile([128, T], c_dt, tag="c", name="cneg_t")
                res = ep.tile([128, T], out_dt, tag="res", name="res_t")
                gm1 = None
                if bneg_dve:
                    gm1 = ep.tile([128, T], pl_dt, tag="gm1", name="gm1_t")
                tw = T // tsplit
                for h in range(tsplit):
                    hs = slice(h * tw, (h + 1) * tw)
                    if bneg_dve:
                        # tail chains: TS(4x)+TT(2x) on DVE beats the Pool
                        # round-trip latency
                        nc.vector.tensor_scalar_add(gm1[:, hs], gs[:, hs], -1.0)
                        nc.vector.tensor_mul(bneg[:, hs], gm1[:, hs], z_ap[:, hs])
                    else:
                        # steady state: one GPSIMD op keeps the DVE free for
                        # scans
                        nc.gpsimd.scalar_tensor_tensor(
                            bneg[:, hs], gs[:, hs], 1.0, z_ap[:, hs],
                            op0=mybir.AluOpType.subtract,
                            op1=mybir.AluOpType.mult,
                        )
                    init = 0.0 if h == 0 else cneg[:, h * tw - 1 : h * tw]
                    nc.vector.tensor_tensor_scan(
                        cneg[:, hs], gs[:, hs], bneg[:, hs], init,
                        op0=mybir.AluOpType.mult, op1=mybir.AluOpType.add,
                    )
                    nc.vector.tensor_mul(res[:, hs], oneg_ap[:, hs], cneg[:, hs])
                    flush_stores()
                    for rs, ds in stores:
                        if tsplit > 1:
                            # kernel tail: store immediately, nothing follows
                            nc.scalar.dma_start(
                                ds[:, hs], res[rs[0] : rs[1], hs]
                            )
                        else:
                            pending_stores.append(
                                (ds[:, hs], res[rs[0] : rs[1], hs])
                            )

            def gemm(p, xt, s, start, stop, tb_major=False):
                """One (slice, row) matmul group: 3 k-chunks x 4 T-blocks
                accumulating into psum p.  tb_major orders T-blocks outermost
                so each 512-column block closes after its 3 k-passes — used
                near the kernel tail so the activations start early."""
                if tb_major:
                    for tb in range(NT):
                        for ki, (k0, kn) in enumerate(KC):
                            nc.tensor.matmul(
                                p[:, bass.ts(tb, TS)],
                                lhsT=w[:kn, ki, s, :],
                                rhs=xt[:kn, ki, bass.ts(tb, TS)],
                                start=start and ki == 0,
                                stop=stop and ki == len(KC) - 1,
                            )
                    return
                for ki, (k0, kn) in enumerate(KC):
                    for tb in range(NT):
                        nc.tensor.matmul(
                            p[:, bass.ts(tb, TS)],
                            lhsT=w[:kn, ki, s, :],
                            rhs=xt[:kn, ki, bass.ts(tb, TS)],
                            start=start and ki == 0,
                            stop=stop and ki == len(KC) - 1,
                        )

            for d_out in d_outs:
              for pair in range(BC // 2):
                b0, b1 = 2 * pair, 2 * pair + 1
                g2 = g2s[pair]
                xts = {}
                gts = {}
                for b in (b0, b1):
                    xt = xp.tile([128, 3, T], mm_dt, tag="x", name="xt_t")
                    # k0/k1 loaded in T-chunks so the first matmuls of each
                    # batch row start sooner (smaller first transfer)
                    nc.sync.dma_start(xt[:, 0, 0:512], d_x[b, 0:128, 0:512])
                    nc.sync.dma_start(xt[:, 0, 512:1024], d_x[b, 0:128, 512:1024])
                    nc.sync.dma_start(xt[:, 0, 1024:T], d_x[b, 0:128, 1024:T])
                    nc.sync.dma_start(xt[:, 1, 0:1024], d_x[b, 128:256, 0:1024])
                    nc.sync.dma_start(xt[:, 1, 1024:T], d_x[b, 128:256, 1024:T])
                    nc.sync.dma_start(xt[:45, 2, :], d_x[b, 256:DP, :])
                    xts[b] = xt
                    gt = gp.tile([128, 2, T], pl_dt, tag="g", name="gt_t")
                    nc.sync.dma_start(
                        gt[:, :, :],
                        d_g[b, 0:256, :].rearrange("(c p) t -> p c t", c=2),
                    )
                    gts[b] = gt
                nc.sync.dma_start(g2[0:44, :], d_g[b0, 256:D, :])
                nc.sync.dma_start(g2[64:108, :], d_g[b1, 256:D, :])

                def do_j(b, j, tsplit=1, bneg_dve=False):
                    m0 = 128 * j
                    pz = ps.tile([128, T], F32, tag="p", name="psum_z")
                    po = ps.tile([128, T], F32, tag="p", name="psum_o")
                    gemm(pz, xts[b], j, start=True, stop=True, tb_major=tsplit > 1)
                    gemm(po, xts[b], 2 + j, start=True, stop=True, tb_major=tsplit > 1)
                    z_j = zp.tile([128, T], pl_dt, tag="z", name="t_z")
                    oneg_j = zp.tile([128, T], pl_dt, tag="o", name="t_o")
                    tw2 = T // tsplit
                    for h in range(tsplit):
                        hs = slice(h * tw2, (h + 1) * tw2)
                        nc.scalar.activation(z_j[:, hs], pz[:, hs], Tanh, scale=1.0)
                        nc.scalar.activation(
                            oneg_j[:, hs], po[:, hs], Tanh, scale=-1.0
                        )
                    chain(
                        gts[b][:, j, :], z_j[:, :], oneg_j[:, :],
                        [((0, 128), d_out[b, m0 : m0 + 128, :])],
                        tsplit=tsplit, bneg_dve=bneg_dve,
                    )

                def rag_gemm_acts(b, z2, oneg2, lanes):
                    """Ragged slice GEMM for row b + repack activations into
                    z2/oneg2 at partition base `lanes` (0 for b_even, 64 for
                    b_odd).  The psum has z at 0:44 and o at 64:108; the
                    zero weight-pad columns make partitions 44:64 / 108:128
                    exact zeros, so the 64-wide activations also initialize
                    the pad lanes (tanh(0)=0) every pair."""
                    pr = ps.tile([128, T], F32, tag="p", name="psum_rag")
                    gemm(pr, xts[b], 4, start=True, stop=True)
                    ls = slice(lanes, lanes + 64)
                    nc.scalar.activation(z2[ls, :], pr[0:64, :], Tanh, scale=1.0)
                    nc.scalar.activation(
                        oneg2[ls, :], pr[64:128, :], Tanh, scale=-1.0
                    )

                z2 = zp.tile([128, T], pl_dt, tag="z", name="t_z2")
                oneg2 = zp.tile([128, T], pl_dt, tag="o", name="t_o2")

                last = pair == BC // 2 - 1
                do_j(b0, 0)
                do_j(b0, 1)
                rag_gemm_acts(b0, z2, oneg2, 0)
                rag_gemm_acts(b1, z2, oneg2, 64)
                # ragged recurrence: both rows of the pair in one 128-lane
                # chain (b0 at 0:44, b1 at 64:108); runs two units before the
                # pair's end so its scan clears the DVE before the tail
                chain(
                    g2[:, :], z2[:, :], oneg2[:, :],
                    [((0, 44), d_out[b0, 256:D, :]),
                     ((64, 108), d_out[b1, 256:D, :])],
                )
                do_j(b1, 0)
                # the kernel's very last chain is split in T-halves so its
                # scan/mul/store pipeline instead of dangling serially
                do_j(b1, 1, tsplit=2 if last else 1, bneg_dve=last)

    nc.compile()
    return nc


def kernel(gate_encoding, inputs_encoding, Wz, bz, Wo, bo):
    gate_encoding = np.asarray(gate_encoding, dtype=np.float32)
    inputs_encoding = np.asarray(inputs_encoding, dtype=np.float32)
    Wz = np.asarray(Wz, dtype=np.float32)
    bz = np.asarray(bz, dtype=np.float32)
    Wo = np.asarray(Wo, dtype=np.float32)
    bo = np.asarray(bo, dtype=np.float32)

    mm_np = np.float16 if CFG["mm16"] else np.float32
    pl_np = np.float16 if CFG["plane16"] else np.float32

    wzo = build_weights(Wz, bz, Wo, bo, mm_np)

    if "nc" not in _CACHE:
        _CACHE["nc"] = _build_program()
    nc = _CACHE["nc"]

    in_maps = []
    for cc in range(NCORES):
        xs = inputs_encoding[cc * BC : (cc + 1) * BC]  # [BC, T, D]
        gs = gate_encoding[cc * BC : (cc + 1) * BC]
        xt = np.empty((BC, DP, T), dtype=mm_np)
        xt[:, :D, :] = xs.transpose(0, 2, 1)
        xt[:, D, :] = 1.0
        gt = gs.transpose(0, 2, 1).astype(pl_np)
        in_maps.append({"xt": xt, "gt": gt, "wzo": wzo})

    res = run_bass_kernel_spmd(nc, in_maps, core_ids=list(range(NCORES)))

    out = np.empty((B, T, D), dtype=np.float32)
    for cc in range(NCORES):
        out[cc * BC : (cc + 1) * BC] = (
            res.results[cc]["outt"].transpose(0, 2, 1).astype(np.float32)
        )
    return out


def build_weights(Wz, bz, Wo, bo, mm_np):
    """Combined [DP, 5*128] operand: slices 0,1 = Wz.T cols 0:256, slices
    2,3 = Wo.T cols 0:256, slice 4 = [Wz.T 256:300 | 0 | Wo.T 256:300 | 0]
    (zero pads keep the ragged psum's unused partitions exactly 0). The
    ones-row (DP-1) carries the biases."""

    def aug(Wmat, bvec):
        a = np.empty((DP, D), dtype=np.float32)
        a[:D, :] = Wmat.T
        a[D, :] = bvec
        return a

    wz_aug = aug(Wz, bz)
    wo_aug = aug(Wo, bo)
    wzo = np.zeros((DP, NS * 128), dtype=np.float32)
    wzo[:, 0:256] = wz_aug[:, 0:256]
    wzo[:, 256:512] = wo_aug[:, 0:256]
    wzo[:, 512:556] = wz_aug[:, 256:300]
    wzo[:, 576:620] = wo_aug[:, 256:300]
    return wzo.astype(mm_np)


# revision 8
# speedup vs baseline: 1.1610x; 1.1610x over previous
"""MRU encoding kernel for Trainium2 (8 NeuronCores, batch-parallel).

Problem (B=32, T=2048, D=300):
    z = tanh(x @ Wz.T + bz); o = tanh(x @ Wo.T + bo)
    c_t = g_t*c_{t-1} + (1-g_t)*z_t   (c_{-1}=0, scan over T)
    out = o * c

Per-core (4 batch rows) layout is [channel, time]:
  - host pre-transposes x,g to [b, D, T]; x gets a ones-row (301) so the
    bias rides in the matmul contraction; the two weight matrices are fed
    as ONE combined [D+1, 5, 128] operand: slices 0,1 = Wz.T columns
    0:256, slices 2,3 = Wo.T columns 0:256, slice 4 = the ragged columns
    of BOTH weights ([Wz.T 256:300 | zeros | Wo.T 256:300 | zeros]) so
    one GEMM per (row, slice) covers them: 15 matmul groups per row
    instead of 18.
  - o is produced NEGATED via tanh(scale=-1): with bneg=(g-1)*z = -(1-g)z
    the hardware scan state=g*state+bneg yields -c, and (-o)*(-c) = o*c.
  - the whole T=2048 recurrence per channel is ONE tensor_tensor_scan
    DVE instruction per 128-channel tile (state kept fp32 by HW).
  - bneg is computed on the (otherwise idle) GPSIMD engine via
    scalar_tensor_tensor so the DVE only runs scans and the final mult.
  - the ragged slice-4 psum holds z at partitions 0:44 and o at 64:108;
    partition-shifted activations repack BOTH batch rows of a pair into
    one 128-lane z2/oneg2 pair (b_even at 0:44, b_odd at 64:108) so the
    ragged recurrence is one DVE chain per pair.  The zero weight-pad
    columns make the activations also rewrite the pad lanes (tanh(0)=0),
    keeping every lane finite without extra memsets.
  - input loads ride the SP HWDGE ring; weights+stores ride the ACT ring
    (HWDGE is FIFO per issuing engine; stores must not block prefetch).
"""

import numpy as np

import concourse.bass as bass
import concourse.mybir as mybir
import concourse.tile as tile
from concourse import bacc
from concourse.bass_utils import run_bass_kernel_spmd

B, T, D = 32, 2048, 300
NCORES = 8
BC = B // NCORES  # 4 batch rows per core
DP = D + 1  # ones-row at index 300 carries the bias
NS = 5  # combined-weight m-slices: z0 z1 o0 o1 ragged
TS = 512  # moving-operand max free dim
NT = T // TS
F32 = mybir.dt.float32
F32R = mybir.dt.float32r
F16 = mybir.dt.float16

KC = [(0, 128), (128, 128), (256, 45)]  # k-chunks (incl. ones row)

CFG = {"mm16": True, "plane16": True, "c16": True, "out16": True}

_CACHE: dict = {}

Tanh = mybir.ActivationFunctionType.Tanh


def _build_program(reps=1, bufs=None, cfg=None):
    c = dict(CFG)
    if cfg:
        c.update(cfg)
    mm_dt = F16 if c["mm16"] else F32R
    pl_dt = F16 if c["plane16"] else F32
    c_dt = F16 if c["c16"] else F32
    out_dt = F16 if c["out16"] else F32

    bf = {"xp": 2, "gp": 2, "zp": 2, "ep": 3, "ps": 2}
    if bufs:
        bf.update(bufs)

    nc = bacc.Bacc("TRN2", target_bir_lowering=False, debug=False, num_devices=NCORES)

    d_x = nc.dram_tensor("xt", [BC, DP, T], mm_dt, kind="ExternalInput").ap()
    d_g = nc.dram_tensor("gt", [BC, D, T], pl_dt, kind="ExternalInput").ap()
    d_w = nc.dram_tensor("wzo", [DP, NS * 128], mm_dt, kind="ExternalInput").ap()
    # replicas share ONE output tensor: keeps the PJRT buffer count (and its
    # per-call overhead) constant across reps so marginal timing is clean
    d_out0 = nc.dram_tensor("outt", [BC, D, T], out_dt, kind="ExternalOutput").ap()
    d_outs = [d_out0] * reps

    with tile.TileContext(nc) as tc:
        with (
            tc.tile_pool(name="wp", bufs=1) as wp,
            tc.tile_pool(name="g2p", bufs=1) as g2p,
            tc.tile_pool(name="xp", bufs=bf["xp"]) as xp,
            tc.tile_pool(name="gp", bufs=bf["gp"]) as gp,
            tc.tile_pool(name="zp", bufs=bf["zp"]) as zp,
            tc.tile_pool(name="ep", bufs=bf["ep"]) as ep,
            tc.tile_pool(name="ps", bufs=bf["ps"], space="PSUM") as ps,
        ):
            # weights ride the scalar ring so they don't delay the first x load
            w = wp.tile([128, 3, NS, 128], mm_dt, tag="w", name="w_t")
            nc.scalar.dma_start(
                w[:, 0:2, :, :],
                d_w[0:256, :].rearrange("(c p) (s m) -> p c s m", c=2, s=NS),
            )
            nc.scalar.dma_start(
                w[:45, 2, :, :], d_w[256:DP, :].rearrange("p (s m) -> p s m", s=NS)
            )

            # persistent ragged-gate tiles (one per pair): pad lanes memset
            # once to a scan-safe finite value, live lanes DMA'd per pair
            g2s = []
            for pr in range(BC // 2):
                g2 = g2p.tile([128, T], pl_dt, tag=f"g2_{pr}", name=f"g2_{pr}")
                nc.gpsimd.memset(g2[32:64, :], 0.5)
                nc.gpsimd.memset(g2[96:128, :], 0.5)
                g2s.append(g2)

            # A store dma_start holds the ACT SEQ while its data-ready sem is
            # pending (cost model: waits precede free(SEQ)), starving the
            # activations behind it.  Deferring each chain's stores until the
            # next chain is issued makes every wait resolve instantly.
            pending_stores: list = []

            def flush_stores():
                for ds, res_ap in pending_stores:
                    # stores ride the ACT HWDGE ring (never block prefetch)
                    nc.scalar.dma_start(ds, res_ap)
                pending_stores.clear()

            def chain(gs, z_ap, oneg_ap, stores, tsplit=1, bneg_dve=False):
                """bneg=(g-1)z -> scan(-c) -> out = (-o)*(-c); stores is a
                list of (res_slice, dram_slice). tsplit>1 pipelines the chain
                in T-chunks (scan chained via `initial`) so the final store
                overlaps the rest -- used for the kernel-tail chain."""
                bneg = ep.tile([128, T], pl_dt, tag="bneg", name="bneg_t")
                cneg = ep.tile([128, T], c_dt, tag="c", name="cneg_t")
                res = ep.tile([128, T], out_dt, tag="res", name="res_t")
                gm1 = None
                if bneg_dve:
                    gm1 = ep.tile([128, T], pl_dt, tag="gm1", name="gm1_t")
                tw = T // tsplit
                for h in range(tsplit):
                    hs = slice(h * tw, (h + 1) * tw)
                    if bneg_dve:
                        # tail chains: TS(4x)+TT(2x) on DVE beats the Pool
                        # round-trip latency
                        nc.vector.tensor_scalar_add(gm1[:, hs], gs[:, hs], -1.0)
                        nc.vector.tensor_mul(bneg[:, hs], gm1[:, hs], z_ap[:, hs])
                    else:
                        # steady state: one GPSIMD op keeps the DVE free for
                        # scans
                        nc.gpsimd.scalar_tensor_tensor(
                            bneg[:, hs], gs[:, hs], 1.0, z_ap[:, hs],
                            op0=mybir.AluOpType.subtract,
                            op1=mybir.AluOpType.mult,
                        )
                    init = 0.0 if h == 0 else cneg[:, h * tw - 1 : h * tw]
                    nc.vector.tensor_tensor_scan(
                        cneg[:, hs], gs[:, hs], bneg[:, hs], init,
                        op0=mybir.AluOpType.mult, op1=mybir.AluOpType.add,
                    )
                    nc.vector.tensor_mul(res[:, hs], oneg_ap[:, hs], cneg[:, hs])
                    flush_stores()
                    for rs, ds in stores:
                        if tsplit > 1:
                            # kernel tail: store immediately, nothing follows
                            nc.scalar.dma_start(
                                ds[:, hs], res[rs[0] : rs[1], hs]
                            )
                        else:
                            pending_stores.append(
                                (ds[:, hs], res[rs[0] : rs[1], hs])
                            )

            def gemm(p, xt, s, start, stop, tb_major=False):
                """One (slice, row) matmul group: 3 k-chunks x 4 T-blocks
                accumulating into psum p.  tb_major orders T-blocks outermost
                so each 512-column block closes after its 3 k-passes — used
                near the kernel tail so the activations start early."""
                if tb_major:
                    for tb in range(NT):
                        for ki, (k0, kn) in enumerate(KC):
                            nc.tensor.matmul(
                                p[:, bass.ts(tb, TS)],
                                lhsT=w[:kn, ki, s, :],
                                rhs=xt[:kn, ki, bass.ts(tb, TS)],
                                start=start and ki == 0,
                                stop=stop and ki == len(KC) - 1,
                            )
                    return
                for ki, (k0, kn) in enumerate(KC):
                    for tb in range(NT):
                        nc.tensor.matmul(
                            p[:, bass.ts(tb, TS)],
                            lhsT=w[:kn, ki, s, :],
                            rhs=xt[:kn, ki, bass.ts(tb, TS)],
                            start=start and ki == 0,
                            stop=stop and ki == len(KC) - 1,
                        )

            for d_out in d_outs:
              for pair in range(BC // 2):
                b0, b1 = 2 * pair, 2 * pair + 1
                g2 = g2s[pair]
                xts = {}
                gts = {}
                for b in (b0, b1):
                    xt = xp.tile([128, 3, T], mm_dt, tag="x", name="xt_t")
                    # k0/k1 loaded in T-chunks so the first matmuls of each
                    # batch row start sooner (smaller first transfer)
                    nc.sync.dma_start(xt[:, 0, 0:512], d_x[b, 0:128, 0:512])
                    nc.sync.dma_start(xt[:, 0, 512:1024], d_x[b, 0:128, 512:1024])
                    nc.sync.dma_start(xt[:, 0, 1024:T], d_x[b, 0:128, 1024:T])
                    nc.sync.dma_start(xt[:, 1, 0:1024], d_x[b, 128:256, 0:1024])
                    nc.sync.dma_start(xt[:, 1, 1024:T], d_x[b, 128:256, 1024:T])
                    nc.sync.dma_start(xt[:45, 2, :], d_x[b, 256:DP, :])
                    xts[b] = xt
                    gt = gp.tile([128, 2, T], pl_dt, tag="g", name="gt_t")
                    nc.sync.dma_start(
                        gt[:, :, :],
                        d_g[b, 0:256, :].rearrange("(c p) t -> p c t", c=2),
                    )
                    gts[b] = gt
                nc.sync.dma_start(g2[0:44, :], d_g[b0, 256:D, :])
                nc.sync.dma_start(g2[64:108, :], d_g[b1, 256:D, :])

                def do_j(b, j, tsplit=1, bneg_dve=False):
                    m0 = 128 * j
                    pz = ps.tile([128, T], F32, tag="p", name="psum_z")
                    po = ps.tile([128, T], F32, tag="p", name="psum_o")
                    gemm(pz, xts[b], j, start=True, stop=True, tb_major=tsplit > 1)
                    gemm(po, xts[b], 2 + j, start=True, stop=True, tb_major=tsplit > 1)
                    z_j = zp.tile([128, T], pl_dt, tag="z", name="t_z")
                    oneg_j = zp.tile([128, T], pl_dt, tag="o", name="t_o")
                    tw2 = T // tsplit
                    for h in range(tsplit):
                        hs = slice(h * tw2, (h + 1) * tw2)
                        nc.scalar.activation(z_j[:, hs], pz[:, hs], Tanh, scale=1.0)
                        nc.scalar.activation(
                            oneg_j[:, hs], po[:, hs], Tanh, scale=-1.0
                        )
                    chain(
                        gts[b][:, j, :], z_j[:, :], oneg_j[:, :],
                        [((0, 128), d_out[b, m0 : m0 + 128, :])],
                        tsplit=tsplit, bneg_dve=bneg_dve,
                    )

                def rag_gemm_acts(b, z2, oneg2, lanes):
                    """Ragged slice GEMM for row b + repack activations into
                    z2/oneg2 at partition base `lanes` (0 for b_even, 64 for
                    b_odd).  The psum has z at 0:44 and o at 64:108; the
                    zero weight-pad columns make partitions 44:64 / 108:128
                    exact zeros, so the 64-wide activations also initialize
                    the pad lanes (tanh(0)=0) every pair."""
                    pr = ps.tile([128, T], F32, tag="p", name="psum_rag")
                    gemm(pr, xts[b], 4, start=True, stop=True)
                    ls = slice(lanes, lanes + 64)
                    nc.scalar.activation(z2[ls, :], pr[0:64, :], Tanh, scale=1.0)
                    nc.scalar.activation(
                        oneg2[ls, :], pr[64:128, :], Tanh, scale=-1.0
                    )

                z2 = zp.tile([128, T], pl_dt, tag="z", name="t_z2")
                oneg2 = zp.tile([128, T], pl_dt, tag="o", name="t_o2")

                last = pair == BC // 2 - 1
                do_j(b0, 0)
                do_j(b0, 1)
                rag_gemm_acts(b0, z2, oneg2, 0)
                do_j(b1, 0)
                rag_gemm_acts(b1, z2, oneg2, 64)
                # ragged recurrence: both rows of the pair in one 128-lane
                # chain (b0 at 0:44, b1 at 64:108)
                chain(
                    g2[:, :], z2[:, :], oneg2[:, :],
                    [((0, 44), d_out[b0, 256:D, :]),
                     ((64, 108), d_out[b1, 256:D, :])],
                )
                # the kernel's very last chain is split in T-halves so its
                # scan/mul/store pipeline instead of dangling serially
                do_j(b1, 1, tsplit=2 if last else 1, bneg_dve=last)

    nc.compile()
    return nc


def kernel(gate_encoding, inputs_encoding, Wz, bz, Wo, bo):
    gate_encoding = np.asarray(gate_encoding, dtype=np.float32)
    inputs_encoding = np.asarray(inputs_encoding, dtype=np.float32)
    Wz = np.asarray(Wz, dtype=np.float32)
    bz = np.asarray(bz, dtype=np.float32)
    Wo = np.asarray(Wo, dtype=np.float32)
    bo = np.asarray(bo, dtype=np.float32)

    mm_np = np.float16 if CFG["mm16"] else np.float32
    pl_np = np.float16 if CFG["plane16"] else np.float32

    wzo = build_weights(Wz, bz, Wo, bo, mm_np)

    if "nc" not in _CACHE:
        _CACHE["nc"] = _build_program()
    nc = _CACHE["nc"]

    in_maps = []
    for cc in range(NCORES):
        xs = inputs_encoding[cc * BC : (cc + 1) * BC]  # [BC, T, D]
        gs = gate_encoding[cc * BC : (cc + 1) * BC]
        xt = np.empty((BC, DP, T), dtype=mm_np)
        xt[:, :D, :] = xs.transpose(0, 2, 1)
        xt[:, D, :] = 1.0
        gt = gs.transpose(0, 2, 1).astype(pl_np)
        in_maps.append({"xt": xt, "gt": gt, "wzo": wzo})

    res = run_bass_kernel_spmd(nc, in_maps, core_ids=list(range(NCORES)))

    out = np.empty((B, T, D), dtype=np.float32)
    for cc in range(NCORES):
        out[cc * BC : (cc + 1) * BC] = (
            res.results[cc]["outt"].transpose(0, 2, 1).astype(np.float32)
        )
    return out


def build_weights(Wz, bz, Wo, bo, mm_np):
    """Combined [DP, 5*128] operand: slices 0,1 = Wz.T cols 0:256, slices
    2,3 = Wo.T cols 0:256, slice 4 = [Wz.T 256:300 | 0 | Wo.T 256:300 | 0]
    (zero pads keep the ragged psum's unused partitions exactly 0). The
    ones-row (DP-1) carries the biases."""

    def aug(Wmat, bvec):
        a = np.empty((DP, D), dtype=np.float32)
        a[:D, :] = Wmat.T
        a[D, :] = bvec
        return a

    wz_aug = aug(Wz, bz)
    wo_aug = aug(Wo, bo)
    wzo = np.zeros((DP, NS * 128), dtype=np.float32)
    wzo[:, 0:256] = wz_aug[:, 0:256]
    wzo[:, 256:512] = wo_aug[:, 0:256]
    wzo[:, 512:556] = wz_aug[:, 256:300]
    wzo[:, 576:620] = wo_aug[:, 256:300]
    return wzo.astype(mm_np)
